# revision 21
# baseline (speedup 1.0000x reference)
"""BinaryLinear kernel for Trainium2, 8 NeuronCores.

y = x @ sign(W)^T + bias
  x: (8, 1024, 4096) f32, W: (4096, 4096) f32, bias: (4096,) f32
  y: (8, 1024, 4096) f32

Data-parallel over batch (8 batches -> 8 cores); each core computes
y_c[1024, 4096] = x_c @ sign(W)^T + b.

Precision scheme (all quantization host-side):
  sign(W) in {-1,0,1} is exact in fp8 e4m3, so the matmul runs in the
  PE's fp8 DoubleRow mode (2x the bf16 rate). x quantized to e4m3 alone
  gives ~2.66e-2 rel err, above the 2e-2 budget; the first 2048
  k-indices therefore also get a correction term xl = e4m3(x - e4m3(x))
  against the same sign weights. Residual error ~2.66e-2 * sqrt(0.5)
  ~= 1.87e-2.

Measured machine model driving the schedule:
  - DR matmul cadence 216 ns (512 moving cols); LDWEIGHTS is emitted
    1:1 with matmuls and fully hidden.
  - DMA transfers deliver ~nothing until t~8us, then ~310 GB/s
    aggregate; each trigger queue is a shallow ring (~3 outstanding),
    so per-queue issue order IS transfer order.
  - The PE clock gate (HAM) opens after ~3.4us of continuous activity
    and drops to half clock on any stall; warm-up matmuls only delay
    real work because the framework preamble blocks the tensor queue
    until ~7us anyway.

Schedule: group 0 (cols 0-1024) is the DMA-critical window and runs as
TWO 512-wide sub-groups, each chunk-major over two m-blocks of 4
tiles: a 128KB w chunk (2 k-subtiles x 512 cols) feeds 4 matmuls, so
the first matmul needs only 256KB of data and the front demand stays
~220 GB/s with multi-us slack. Each pass holds 4 open psum chains
(1 bank each); the 8-bank rotation lets consecutive passes pipeline.
All front loads are issued in strict need order, round-robined across
the sync/scalar/gpsimd trigger queues so no queue races ahead.
Groups 1-3 (w resident/prefetched) run m-major, which measures
gap-free. 12 output buffers decouple evictions from y-write DMA
latency. The final m-tile staggers its two psum chains and splits the
last eviction DMA across 2 queues.
"""

import numpy as np
import ml_dtypes

import concourse.bass as bass
import concourse.tile as tile
from concourse import bacc, mybir
from concourse.bass_utils import run_bass_kernel_spmd

B, S, DIN, DOUT = 8, 1024, 4096, 4096
P = 128
KT = DIN // P          # 32 k-subtiles
KPAIR = KT // 2        # 16 hi pair-tiles
MT = S // P            # 8 m tiles
NF = 512               # psum bank width fp32
GW = 1024              # n-group width for m-major groups (2 psum banks)
NG = DOUT // GW        # 4 groups

KCP = 8                # correction pair-tiles (fraction 2*KCP/KT of k)
KTC = 2 * KCP          # correction k-subtiles
KC = KTC * P           # corrected k indices
XCH = 8                # k-subtiles per x DMA chunk
NXC = KT // XCH        # x chunks per m tile
WCHUNK = 2             # k-subtiles per w DMA chunk
NWC = KT // WCHUNK     # w chunks per group

N_CORES = 8
DR = mybir.MatmulPerfMode.DoubleRow

E4 = ml_dtypes.float8_e4m3   # TRN FP8_EXP4-compatible (max 240)


def build_nc():
    nc = bacc.Bacc("TRN2", target_bir_lowering=False, debug=False,
                   num_devices=N_CORES)

    xq = nc.dram_tensor("xq", [MT, P, KT, P], mybir.dt.float8e4,
                        kind="ExternalInput")
    wq = nc.dram_tensor("wq", [DIN, DOUT], mybir.dt.float8e4,
                        kind="ExternalInput")
    bias = nc.dram_tensor("bias", [P, DOUT], mybir.dt.float32,
                          kind="ExternalInput")
    xl = nc.dram_tensor("xl", [MT, P, KTC, P], mybir.dt.float8e4,
                        kind="ExternalInput")
    y = nc.dram_tensor("y", [S, DOUT], mybir.dt.float32, kind="ExternalOutput")

    xq_ap = xq.ap()
    xl_ap = xl.ap()
    wq_r = wq.ap().rearrange("(k p) o -> p k o", p=P)   # [128, 32, 4096]
    y_ap = y.ap()
    bias_ap = bias.ap()

    with tile.TileContext(nc) as tc:
        with (
            tc.tile_pool(name="xpool", bufs=1) as xpool,
            tc.tile_pool(name="bpool", bufs=1) as bpool,
            tc.tile_pool(name="w0pool", bufs=1) as w0pool,
            tc.tile_pool(name="gpool", bufs=1) as gpool,
            tc.tile_pool(name="wpool", bufs=2) as wpool,
            tc.tile_pool(name="opool", bufs=6) as opool,
            tc.tile_pool(name="psum", bufs=8, space="PSUM") as psum,
        ):
            def load_w0(sub, c, eng):
                """512-wide w chunk for group-0 sub-group `sub`."""
                t = w0pool.tile([P, WCHUNK, NF], mybir.dt.float8e4,
                                name=f"w0{sub}_{c}", tag=f"w0{sub}_{c}")
                eng.dma_start(
                    t[:],
                    wq_r[:, c * WCHUNK:(c + 1) * WCHUNK,
                         sub * NF:(sub + 1) * NF])
                return t

            def load_w_chunk(g, c, eng):
                t = wpool.tile([P, WCHUNK, GW], mybir.dt.float8e4,
                               name=f"w_{c}", tag=f"w_{c}")
                eng.dma_start(
                    t[:],
                    wq_r[:, c * WCHUNK:(c + 1) * WCHUNK,
                         g * GW:(g + 1) * GW])
                return t

            def load_w_group(g):
                return [load_w_chunk(g, c, nc.sync) for c in range(NWC)]

            def wslice(chunks, kt2, h):
                """rhs pair AP for k-subtiles (kt2, kt2+1), n-half h."""
                c = kt2 // WCHUNK
                return chunks[c][:, :, h * NF:(h + 1) * NF]

            def load_x_chunk(m, c, eng):
                t = xpool.tile([P, XCH, P], mybir.dt.float8e4,
                               name=f"xq_{m}_{c}", tag=f"xq_{m}_{c}")
                eng.dma_start(t[:], xq_ap[m, :, c * XCH:(c + 1) * XCH, :])
                return t

            def load_xl(m, eng):
                t = xpool.tile([P, KTC, P], mybir.dt.float8e4,
                               name=f"xl_{m}", tag=f"xl_{m}")
                eng.dma_start(t[:], xl_ap[m])
                return t

            def load_bias(g, eng):
                t = bpool.tile([P, GW], mybir.dt.float32,
                               name=f"bias_{g}", tag=f"bias_{g}")
                eng.dma_start(t[:], bias_ap[:, g * GW:(g + 1) * GW])
                return t

            xq_t = [[None] * NXC for _ in range(MT)]
            xl_t = [None] * MT
            bias_t = [None] * NG
            w0a = [None] * NWC
            w0b = [None] * NWC

            # --- group-0 front: strict need order, round-robin across the
            # three trigger queues (shallow rings make per-queue order =
            # transfer order; RR keeps the queues in lockstep).
            rr_engs = [nc.sync, nc.scalar, nc.gpsimd]
            rr_i = [0]

            def rr():
                e = rr_engs[rr_i[0] % 3]
                rr_i[0] += 1
                return e

            w0a[0] = load_w0(0, 0, rr())
            for m in range(MT):
                xq_t[m][0] = load_x_chunk(m, 0, rr())
            for c in (1, 2, 3):
                w0a[c] = load_w0(0, c, rr())
            for xc in (1, 2, 3):
                for m in range(MT):
                    xq_t[m][xc] = load_x_chunk(m, xc, rr())
                for c in range(4 * xc, 4 * xc + 4):
                    w0a[c] = load_w0(0, c, rr())
            for c in range(NWC):
                w0b[c] = load_w0(1, c, rr())
            for m in range(MT):
                xl_t[m] = load_xl(m, rr())
            bias_t[0] = load_bias(0, rr())

            def evict(pt, m, g, h):
                ot = opool.tile([P, NF], mybir.dt.float32, name="ot", tag="ot")
                nc.vector.tensor_add(
                    ot[:], pt[:], bias_t[g][:, h * NF:(h + 1) * NF])
                nc.scalar.dma_start(
                    y_ap[m * P:(m + 1) * P,
                         g * GW + h * NF:g * GW + (h + 1) * NF], ot[:])

            def evict_final(pt, m, g, h):
                """last eviction: 256-col pieces, DMA split over 2 queues
                so the tail transfer overlaps the remaining compute."""
                ot = opool.tile([P, NF], mybir.dt.float32, name="ot", tag="ot")
                engs = [nc.scalar, nc.sync]
                for q in range(2):
                    sl = slice(q * 256, (q + 1) * 256)
                    nc.vector.tensor_add(
                        ot[:, sl], pt[:, sl],
                        bias_t[g][:, h * NF + q * 256:h * NF + (q + 1) * 256])
                    engs[q].dma_start(
                        y_ap[m * P:(m + 1) * P,
                             g * GW + h * NF + q * 256:
                             g * GW + h * NF + (q + 1) * 256], ot[:, sl])

            # --- group 0: two 512-wide sub-groups, four 4-m hi sweeps
            # in A/A'/B/B' order. A' (m0-3 on w0b) reuses the resident
            # xqA, so the w-demand and x-demand windows alternate instead
            # of stacking; uncorrected outputs are held in SBUF (scalar
            # copies), and the correction sweeps + bias add + y writes run
            # after all sweeps, when xl/bias have long arrived.
            held = {}

            def g0_sweep(sub, wlist, mlist, mid_hook=None):
                if mid_hook is not None:
                    mid_hook()
                pts = {m: psum.tile([P, NF], mybir.dt.float32,
                                    name="pt", tag="pt") for m in mlist}
                for c in range(NWC):
                    xc, r = divmod(WCHUNK * c, XCH)
                    for m in mlist:
                        nc.tensor.matmul(
                            pts[m][:], xq_t[m][xc][:, r:r + 2, :],
                            wlist[c][:], start=(c == 0), stop=(c == NWC - 1),
                            perf_mode=DR)
                for m in mlist:
                    ot = gpool.tile([P, NF], mybir.dt.float32,
                                    name=f"og_{m}_{sub}", tag=f"og_{m}_{sub}")
                    nc.scalar.copy(ot[:], pts[m][:])
                    held[(m, sub)] = ot

            def g0_corr(sub, wlist):
                for m in range(MT):
                    cp = psum.tile([P, NF], mybir.dt.float32,
                                   name="pt", tag="pt")
                    for kq in range(KCP):
                        nc.tensor.matmul(
                            cp[:], xl_t[m][:, 2 * kq:2 * kq + 2, :],
                            wlist[kq][:], start=(kq == 0),
                            stop=(kq == KCP - 1), perf_mode=DR)
                    ot = held[(m, sub)]
                    nc.vector.tensor_add(ot[:], ot[:], cp[:])
                    nc.vector.tensor_add(
                        ot[:], ot[:], bias_t[0][:, sub * NF:(sub + 1) * NF])
                    nc.scalar.dma_start(
                        y_ap[m * P:(m + 1) * P,
                             sub * NF:(sub + 1) * NF], ot[:])

            w_next = []
            bias_next = []

            def prefetch_g1():
                w_next.append(load_w_group(1))
                bias_next.append(load_bias(1, nc.gpsimd))

            allm = list(range(MT))
            g0_sweep(0, w0a, allm)
            g0_sweep(1, w0b, allm, mid_hook=prefetch_g1)
            g0_corr(0, w0a)
            g0_corr(1, w0b)
            w_cur = w_next[0]
            bias_t[1] = bias_next[0]

            # --- groups 1-3: m-major, w prefetched during previous group.
            def chain(pts, m, hi, h):
                """full k accumulation chain for psum half h of m-tile m."""
                for kp in range(KPAIR):
                    xc, xr = divmod(2 * kp, XCH)
                    nc.tensor.matmul(
                        pts[h][:], xq_t[m][xc][:, xr:xr + 2, :],
                        wslice(hi, 2 * kp, h),
                        start=(kp == 0), stop=False, perf_mode=DR)
                for kq in range(KCP):
                    nc.tensor.matmul(
                        pts[h][:], xl_t[m][:, 2 * kq:2 * kq + 2, :],
                        wslice(hi, 2 * kq, h),
                        start=False, stop=(kq == KCP - 1), perf_mode=DR)

            for g in range(1, NG):
                hi = w_cur
                for m in range(MT):
                    pts = [psum.tile([P, NF], mybir.dt.float32,
                                     name="pt", tag="pt")
                           for _ in range(2)]
                    if g == NG - 1 and m == MT - 1:
                        # tail stagger: finish half 0 first so its eviction
                        # and y DMA overlap half 1's matmuls.
                        chain(pts, m, hi, 0)
                        evict(pts[0], m, g, 0)
                        chain(pts, m, hi, 1)
                        evict_final(pts[1], m, g, 1)
                        continue
                    # interleave halves: h0/h1 share each stationary x pair
                    for kp in range(KPAIR):
                        xc, xr = divmod(2 * kp, XCH)
                        lhsT = xq_t[m][xc][:, xr:xr + 2, :]
                        for h in range(2):
                            nc.tensor.matmul(
                                pts[h][:], lhsT, wslice(hi, 2 * kp, h),
                                start=(kp == 0), stop=False, perf_mode=DR)
                    for kq in range(KCP):
                        lhsT = xl_t[m][:, 2 * kq:2 * kq + 2, :]
                        for h in range(2):
                            nc.tensor.matmul(
                                pts[h][:], lhsT, wslice(hi, 2 * kq, h),
                                start=False, stop=(kq == KCP - 1),
                                perf_mode=DR)
                    if m == 5 and g + 1 < NG:
                        w_nx = load_w_group(g + 1)
                    if m == 6 and g + 1 < NG:
                        bias_t[g + 1] = load_bias(g + 1, nc.gpsimd)
                    for h in range(2):
                        evict(pts[h], m, g, h)
                if g + 1 < NG:
                    w_cur = w_nx

    nc.compile()
    return nc


def _prep_inputs(x, weight, bias):
    x = np.asarray(x, dtype=np.float32)
    weight = np.asarray(weight, dtype=np.float32)
    bias = np.asarray(bias, dtype=np.float32)

    sg = np.sign(weight).T                         # [DIN, DOUT]
    wq = np.ascontiguousarray(sg).astype(E4)
    xq8 = x.astype(E4)
    # [b, s, i] -> [b, m, p_i, k, p_s]
    xq = np.ascontiguousarray(
        xq8.reshape(B, MT, P, KT, P).transpose(0, 1, 4, 3, 2))
    r = x[..., :KC] - xq8[..., :KC].astype(np.float32)
    xl8 = r.astype(E4)
    xl = np.ascontiguousarray(
        xl8.reshape(B, MT, P, KTC, P).transpose(0, 1, 4, 3, 2))
    bias_bc = np.ascontiguousarray(np.broadcast_to(bias[None, :], (P, DOUT)))
    return {"xq": xq, "wq": wq, "bias": bias_bc, "xl": xl}


_NC_CACHE = []


def kernel(x, weight, bias, _trace=False):
    ins = _prep_inputs(x, weight, bias)

    if not _NC_CACHE:
        _NC_CACHE.append(build_nc())
    nc = _NC_CACHE[0]
    core_ids = list(range(N_CORES))
    in_maps = [{k: (v[c] if k in ("xq", "xl") else v)
                for k, v in ins.items()} for c in core_ids]
    res = run_bass_kernel_spmd(nc, in_maps, core_ids, trace=_trace)

    out = np.empty((B, S, DOUT), dtype=np.float32)
    for c in core_ids:
        out[c] = res.results[c]["y"]
    if _trace:
        kernel.last_result = res
    return out
